# revision 13
# baseline (speedup 1.0000x reference)
"""DeepEMD kernel for 8x Trainium2 NeuronCores (Bass/Tile).

Computes (logits, weight_1, weight_2) of the DeepEMD head:
  weight_1 = relu(einsum('mchw,nc->mnhw', query, GAP(proto))) + 0.001
  weight_2 = relu(einsum('nchw,mc->nmhw', proto, GAP(query))) + 0.001
  sim      = cosine sim over channels of centered features  [M,N,HW,HW]
  logits   = ((1-sim)*flow).sum((-1,-2)) * (T/HW)

Sharding: M (query rows) split across 8 cores; proto replicated.

Device strategy (per core):
  Host packs qe = (q^T*rsq) [640,784] bf16 and pe = (p^T*rsp) [640,6272]
  bf16 so psum[ip,jq] = qe^T@pe = dot_raw*rsq*rsp.  With the identity
     sum_pq flow*(1-sim) = F3 - sum_pq flow*dot_raw*rsq*rsp,
     F3[i,j] = sum_pq flow*(1 + C*qm*pm*rsq*rsp)   (host, tiny),
  the device only needs the raw 640-deep contraction, a bf16 multiply
  by flow (DVE 2x) and a grouped q-reduce; p-sums + F3 finish on host.
  weight_1/weight_2 are small fp16 matmuls with fused max(x+1e-3, 1e-3).
"""

import sys

if "/opt/trn_rl_repo" not in sys.path:
    sys.path.insert(0, "/opt/trn_rl_repo")

import ml_dtypes
import numpy as np

import concourse.bass as bass
import concourse.tile as tile
from concourse import mybir
from concourse.bass_utils import run_bass_kernel_spmd

# Problem shape (hardcoded per contest rules)
M = 128          # queries
N = 128          # protos
C = 640          # channels
HW = 49          # spatial
NCORES = 8
MS = M // NCORES          # 16 queries per core
IP = MS * HW              # 784 query rows (i,p) per core
JQ = N * HW               # 6272 proto cols (j,q)
KC = C // 128             # 5 contraction chunks of 128
TEMP = 12.5
COS_EPS = 1e-8

CHUNK = 490               # 10 j's per N-chunk (q-group aligned)
CHUNK_W = [CHUNK] * 12 + [JQ - 12 * CHUNK]     # 13 chunks
M_TILES = [128] * 6 + [IP - 6 * 128]           # 784 = 6*128 + 16

# pe is DMA'd in 4 column blocks (chunk-aligned) so the sim loop can
# start as soon as block 0 lands
PE_BLK_CH = [4, 3, 3, 3]                       # chunks per block
PE_BLK_START = [0, 4, 7, 10]
PE_BLK_COL = [0, 4 * CHUNK, 7 * CHUNK, 10 * CHUNK]
PE_BLK_W = [4 * CHUNK, 3 * CHUNK, 3 * CHUNK, JQ - 10 * CHUNK]

BF16 = mybir.dt.bfloat16
FP16 = mybir.dt.float16
F32 = mybir.dt.float32

NP_BF16 = ml_dtypes.bfloat16
NP_FP16 = np.float16

_BUILT = {}
LAST_RESULT = None        # test harness reads exec_time_ns from here

# walrus embeds sem-waits directly in the ISA structs, which have very few
# wait slots. Tile can attach more than that. Spill the excess onto
# standalone InstEventSemaphore waits inserted just before the offending
# instruction on the same engine stream.
_WAIT_LIMIT = 1


def _split_waits(nc):
    n_split = 0
    for fn in nc.m.functions:
        for blk in fn.blocks:
            out = []
            for inst in blk.instructions:
                si = inst.sync_info
                tn = type(inst).__name__
                if si is not None and tn != "InstEventSemaphore":
                    waits = list(si.on_wait)
                    if len(waits) > _WAIT_LIMIT:
                        spill, keep = waits[:-_WAIT_LIMIT], waits[-_WAIT_LIMIT:]
                        for w in spill:
                            n_split += 1
                            out.append(mybir.InstEventSemaphore(
                                name=f"{inst.name}-wsp{n_split}",
                                engine=inst.engine,
                                sync_info=mybir.SyncInfo(on_wait=[w], on_update=[]),
                            ))
                        inst.sync_info = mybir.SyncInfo(
                            on_wait=keep, on_update=list(si.on_update)
                        )
                out.append(inst)
            blk.instructions = out
    return n_split


def _build_nc():
    nc = bass.Bass()

    d_qe = nc.declare_dram_parameter("qe", [C, IP], BF16, isOutput=False)
    d_pe = nc.declare_dram_parameter("pe", [C, JQ], BF16, isOutput=False)
    d_ft = nc.declare_dram_parameter("ft", [IP, JQ], BF16, isOutput=False)
    d_qf = nc.declare_dram_parameter("qf", [C, IP], FP16, isOutput=False)
    d_pf = nc.declare_dram_parameter("pf", [C, JQ], FP16, isOutput=False)
    d_pmg = nc.declare_dram_parameter("pmg", [C, N], FP16, isOutput=False)
    d_qmg = nc.declare_dram_parameter("qmg", [C, MS], FP16, isOutput=False)

    d_rq = nc.declare_dram_parameter("rq", [128, 7 * N], F32, isOutput=True)
    d_w1t = nc.declare_dram_parameter("w1t", [N, IP], F32, isOutput=True)
    d_w2t = nc.declare_dram_parameter("w2t", [MS, JQ], F32, isOutput=True)

    AOT = mybir.AluOpType
    AFT = mybir.ActivationFunctionType

    with tile.TileContext(nc) as tc:
        with (
            tc.tile_pool(name="const", bufs=1) as cp,
            tc.tile_pool(name="ftp", bufs=2) as ftp,
            tc.tile_pool(name="ev", bufs=3) as evp,
            tc.tile_pool(name="pr", bufs=3) as prp,
            tc.tile_pool(name="wst", bufs=2) as wstp,
            tc.tile_pool(name="psim", bufs=3, space=bass.MemorySpace.PSUM) as psp,
            tc.tile_pool(name="pw1", bufs=2, space=bass.MemorySpace.PSUM) as w1psp,
            tc.tile_pool(name="pw2", bufs=2, space=bass.MemorySpace.PSUM) as w2psp,
        ):
            # ---- DMAs, in priority order ----
            qf_t = []
            pmg_t = []
            for k in range(KC):
                t = cp.tile([128, IP], FP16, tag=f"qf{k}")
                nc.sync.dma_start(t[:], d_qf[k * 128:(k + 1) * 128, :])
                qf_t.append(t)
            for k in range(KC):
                t = cp.tile([128, N], FP16, tag=f"pmg{k}")
                nc.sync.dma_start(t[:], d_pmg[k * 128:(k + 1) * 128, :])
                pmg_t.append(t)

            qe_t = []
            for k in range(KC):
                t = cp.tile([128, IP], BF16, tag=f"qe{k}")
                nc.sync.dma_start(t[:], d_qe[k * 128:(k + 1) * 128, :])
                qe_t.append(t)

            # pe column blocks; block 0 first, then first flow tile, then rest
            pe_t = [[None] * 4 for _ in range(KC)]   # pe_t[k][b]
            for b in range(4):
                for k in range(KC):
                    t = cp.tile([128, PE_BLK_W[b]], BF16, tag=f"pe{k}b{b}")
                    nc.sync.dma_start(
                        t[:],
                        d_pe[k * 128:(k + 1) * 128,
                             PE_BLK_COL[b]:PE_BLK_COL[b] + PE_BLK_W[b]],
                    )
                    pe_t[k][b] = t
                if b == 0:
                    ft0 = ftp.tile([128, JQ], BF16, tag="ft")
                    nc.sync.dma_start(ft0[0:M_TILES[0], :], d_ft[0:M_TILES[0], :])

            # persistent outputs in SBUF
            rq_sb = cp.tile([128, 7 * N], F32, tag="rq")
            w1t_sb = cp.tile([N, IP], F32, tag="w1t")

            # ---- weight_1^T = relu(pmg^T @ qf) + 1e-3 : [N, IP] ----
            for c0 in range(0, IP, 512):
                w = min(512, IP - c0)
                ps = w1psp.tile([128, 512], F32, tag="w1ps")
                for k in range(KC):
                    nc.tensor.matmul(
                        ps[:, 0:w],
                        pmg_t[k][:],
                        qf_t[k][:, c0:c0 + w],
                        start=(k == 0),
                        stop=(k == KC - 1),
                    )
                nc.vector.tensor_scalar(
                    w1t_sb[:, c0:c0 + w], ps[:N, 0:w], 0.001, 0.001,
                    op0=AOT.add, op1=AOT.max,
                )
            nc.gpsimd.dma_start(d_w1t[:, :], w1t_sb[:])

            # ---- main loop: D[ip, j] = sum_q flow * (dot_raw*rsq*rsp) ----
            row0 = 0
            for mt, rows in enumerate(M_TILES):
                if mt == 0:
                    ft_tile = ft0
                else:
                    ft_tile = ftp.tile([128, JQ], BF16, tag="ft")
                    nc.sync.dma_start(
                        ft_tile[0:rows, :], d_ft[row0:row0 + rows, :]
                    )
                for ch, w in enumerate(CHUNK_W):
                    nj = w // HW
                    b = max(i for i in range(4) if PE_BLK_START[i] <= ch)
                    lo = (ch - PE_BLK_START[b]) * CHUNK
                    ps = psp.tile([128, CHUNK], F32, tag="ps")
                    for k in range(KC):
                        nc.tensor.matmul(
                            ps[0:rows, 0:w],
                            qe_t[k][:, row0:row0 + rows],
                            pe_t[k][b][:, lo:lo + w],
                            start=(k == 0),
                            stop=(k == KC - 1),
                        )
                    ev = evp.tile([128, CHUNK], BF16, tag="ev")
                    nc.scalar.activation(ev[0:rows, 0:w], ps[0:rows, 0:w], AFT.Copy)
                    pr_tile = prp.tile([128, CHUNK], BF16, tag="pr")
                    nc.vector.tensor_tensor(
                        pr_tile[0:rows, 0:w],
                        ev[0:rows, 0:w],
                        ft_tile[0:rows, ch * CHUNK:ch * CHUNK + w],
                        op=AOT.mult,
                    )
                    nc.vector.tensor_reduce(
                        rq_sb[0:rows, mt * N + ch * 10: mt * N + ch * 10 + nj],
                        pr_tile[0:rows, 0:w].rearrange("p (j q) -> p j q", q=HW),
                        axis=mybir.AxisListType.X,
                        op=AOT.add,
                    )
                row0 += rows

            # ---- weight_2^T = relu(qmg^T @ pf) + 1e-3 : [MS, JQ] ----
            qmg_t = []
            pf_t = []
            for k in range(KC):
                t = cp.tile([128, MS], FP16, tag=f"qmg{k}")
                nc.sync.dma_start(t[:], d_qmg[k * 128:(k + 1) * 128, :])
                qmg_t.append(t)
            for k in range(KC):
                t = cp.tile([128, JQ], FP16, tag=f"pf{k}")
                nc.sync.dma_start(t[:], d_pf[k * 128:(k + 1) * 128, :])
                pf_t.append(t)

            for c0 in range(0, JQ, 512):
                w = min(512, JQ - c0)
                ps = w2psp.tile([128, 512], F32, tag="w2ps")
                for k in range(KC):
                    nc.tensor.matmul(
                        ps[0:MS, 0:w],
                        qmg_t[k][:],
                        pf_t[k][:, c0:c0 + w],
                        start=(k == 0),
                        stop=(k == KC - 1),
                    )
                ws = wstp.tile([MS, 512], F32, tag="wst")
                nc.vector.tensor_scalar(
                    ws[:, 0:w], ps[0:MS, 0:w], 0.001, 0.001,
                    op0=AOT.add, op1=AOT.max,
                )
                nc.gpsimd.dma_start(d_w2t[:, c0:c0 + w], ws[:, 0:w])
            nc.gpsimd.dma_start(d_rq[:, :], rq_sb[:])

    _split_waits(nc)
    return nc


def _get_nc():
    if "nc" not in _BUILT:
        _BUILT["nc"] = _build_nc()
    return _BUILT["nc"]


def kernel(query, proto, flow):
    global LAST_RESULT
    query = np.asarray(query, dtype=np.float32)
    proto = np.asarray(proto, dtype=np.float32)
    flow = np.asarray(flow, dtype=np.float32)

    qr = query.reshape(M, C, HW)
    pr = proto.reshape(N, C, HW)

    # channel-centering stats (match reference fp32 math)
    qm = qr.mean(axis=1)                      # [M, HW]
    pm = pr.mean(axis=1)                      # [N, HW]
    qc = qr - qm[:, None, :]
    pc = pr - pm[:, None, :]
    nq = np.sqrt((qc * qc).sum(axis=1))       # [M, HW]
    npn = np.sqrt((pc * pc).sum(axis=1))      # [N, HW]
    rsq = 1.0 / np.maximum(nq, COS_EPS)
    rsp = 1.0 / np.maximum(npn, COS_EPS)

    Bp = pr.mean(axis=2)                      # [N, C]  GAP(proto)
    Ap = qr.mean(axis=2)                      # [M, C]  GAP(query)

    # packed contraction operands: psum = qe^T @ pe = dot_raw*rsq*rsp
    qe = np.ascontiguousarray(
        (qr * rsq[:, None, :]).transpose(1, 0, 2).reshape(C, M * HW)
        .astype(NP_BF16))
    pe = np.ascontiguousarray(
        (pr * rsp[:, None, :]).transpose(1, 0, 2).reshape(C, JQ)
        .astype(NP_BF16))

    # F3[i,j] = sum_pq flow * (1 + C*qm*pm*rsq*rsp)
    a_w = C * qm * rsq                        # [M, HW]
    b_w = pm * rsp                            # [N, HW]
    F1 = flow.sum(axis=(2, 3), dtype=np.float64)                     # [M, N]
    Fc = np.einsum('ijpq,ip,jq->ij', flow, a_w, b_w, optimize=True)  # [M, N]
    F3 = F1 + Fc

    qf_full = qr.transpose(1, 0, 2).reshape(C, M * HW).astype(NP_FP16)
    pf = np.ascontiguousarray(pr.transpose(1, 0, 2).reshape(C, JQ).astype(NP_FP16))
    pmg = np.ascontiguousarray(Bp.T.astype(NP_FP16))
    qmg_full = Ap.T.astype(NP_FP16)           # [C, M]

    in_maps = []
    for cc in range(NCORES):
        cols = slice(cc * IP, (cc + 1) * IP)
        ft_cc = (
            flow[cc * MS:(cc + 1) * MS]
            .transpose(0, 2, 1, 3)
            .astype(NP_BF16)
            .reshape(IP, JQ)
        )
        in_maps.append({
            "qe": np.ascontiguousarray(qe[:, cols]),
            "pe": pe,
            "ft": ft_cc,
            "qf": np.ascontiguousarray(qf_full[:, cols]),
            "pf": pf,
            "pmg": pmg,
            "qmg": np.ascontiguousarray(qmg_full[:, cc * MS:(cc + 1) * MS]),
        })

    nc = _get_nc()
    res = run_bass_kernel_spmd(nc, in_maps, list(range(NCORES)))
    LAST_RESULT = res

    logits = np.empty((M, N), np.float32)
    w1 = np.empty((M, N, HW), np.float32)
    w2 = np.empty((N, M, HW), np.float32)
    for cc in range(NCORES):
        out = res.results[cc]
        rq = out["rq"]
        big = np.zeros((IP, N), np.float64)
        r0 = 0
        for mt, rows in enumerate(M_TILES):
            big[r0:r0 + rows] = rq[:rows, mt * N:(mt + 1) * N]
            r0 += rows
        D = big.reshape(MS, HW, N).sum(axis=1)          # [MS, N]
        logits[cc * MS:(cc + 1) * MS] = (
            (F3[cc * MS:(cc + 1) * MS] - D) * (TEMP / HW)
        ).astype(np.float32)
        w1[cc * MS:(cc + 1) * MS] = (
            out["w1t"].reshape(N, MS, HW).transpose(1, 0, 2)
        )
        w2[:, cc * MS:(cc + 1) * MS, :] = (
            out["w2t"].reshape(MS, N, HW).transpose(1, 0, 2)
        )
    return logits, w1, w2


# revision 22
# speedup vs baseline: 24.0706x; 24.0706x over previous
"""DeepEMD kernel for 8x Trainium2 NeuronCores (Bass/Tile).

Computes (logits, weight_1, weight_2) of the DeepEMD head:
  weight_1 = relu(einsum('mchw,nc->mnhw', query, GAP(proto))) + 0.001
  weight_2 = relu(einsum('nchw,mc->nmhw', proto, GAP(query))) + 0.001
  sim      = cosine sim over channels of centered features  [M,N,HW,HW]
  logits   = ((1-sim)*flow).sum((-1,-2)) * (T/HW)

Sharding: M (query rows) split across 8 cores; proto replicated.

Device strategy (per core):
  Host packs qe = (q^T*rsq) [640,784] bf16 and pe = (p^T*rsp) [640,6272]
  bf16 so psum[ip,jq] = qe^T@pe = dot_raw*rsq*rsp.  With the identity
     sum_pq flow*(1-sim) = F3 - sum_pq flow*dot_raw*rsq*rsp,
     F3[i,j] = sum_pq flow*(1 + C*qm*pm*rsq*rsp)   (host, tiny),
  the device only needs the raw 640-deep contraction, one fused
  DVE multiply-by-flow straight out of PSUM (3 banks per op), and a
  grouped q-reduce; p-sums + F3 finish on host.
  weight_1/weight_2 are small fp16 matmuls with fused max(x+1e-3, 1e-3).

  Empirical notes for this environment (measured via in-NEFF repeat
  loops): PE matmuls run at warm spec (~N/2.4GHz), DMA ~280 GB/s/core,
  but every DVE/ACT instruction and its semaphore waits carry multi-us
  overheads -> the design minimizes instruction count (batched 3-bank
  PSUM consumers, one DMA per input tensor).
"""

import os
import sys

if "/opt/trn_rl_repo" not in sys.path:
    sys.path.insert(0, "/opt/trn_rl_repo")

# NTFF tracing is unavailable in this container (antenv.axon_hooks missing);
# make sure run_bass_kernel_spmd never takes the trace path even if the
# caller's environment sets BASS_TRACE.
os.environ["BASS_NEVER_TRACE"] = "1"

import ml_dtypes
import numpy as np

import concourse.bass as bass
import concourse.tile as tile
from concourse import mybir
from concourse.bass_utils import run_bass_kernel_spmd

# Problem shape (hardcoded per contest rules)
M = 128          # queries
N = 128          # protos
C = 640          # channels
HW = 49          # spatial
NCORES = 8
MS = M // NCORES          # 16 queries per core
IP = MS * HW              # 784 query rows (i,p) per core
JQ = N * HW               # 6272 proto cols (j,q)
KC = C // 128             # 5 contraction chunks of 128
TEMP = 12.5
COS_EPS = 1e-8

CHUNK = 490               # 10 j's per psum bank (q-group aligned)
# chunk groups: each group is one [128, g, 512] psum tile (g banks), with
# one fused DVE multiply + one grouped reduce per group
GROUPS = [(0, 3), (3, 3), (6, 3), (9, 3), (12, 1)]   # (chunk0, nchunks)
M_TILES = [128] * 6 + [IP - 6 * 128]                 # 784 = 6*128 + 16

BF16 = mybir.dt.bfloat16
FP8 = mybir.dt.float8e4
FP16 = mybir.dt.float16
F32 = mybir.dt.float32

NP_BF16 = ml_dtypes.bfloat16
NP_FP16 = np.float16
NP_FP8 = ml_dtypes.float8_e4m3

_BUILT = {}
LAST_RESULT = None        # test harness reads exec_time_ns from here

# walrus embeds sem-waits directly in the ISA structs, which have very few
# wait slots. Tile can attach more than that. Spill the excess onto
# standalone InstEventSemaphore waits inserted just before the offending
# instruction on the same engine stream.
_WAIT_LIMIT = 1


def _split_waits(nc):
    n_split = 0
    for fn in nc.m.functions:
        for blk in fn.blocks:
            out = []
            for inst in blk.instructions:
                si = inst.sync_info
                tn = type(inst).__name__
                if si is not None and tn != "InstEventSemaphore":
                    waits = list(si.on_wait)
                    if len(waits) > _WAIT_LIMIT:
                        spill, keep = waits[:-_WAIT_LIMIT], waits[-_WAIT_LIMIT:]
                        for w in spill:
                            n_split += 1
                            out.append(mybir.InstEventSemaphore(
                                name=f"{inst.name}-wsp{n_split}",
                                engine=inst.engine,
                                sync_info=mybir.SyncInfo(on_wait=[w], on_update=[]),
                            ))
                        inst.sync_info = mybir.SyncInfo(
                            on_wait=keep, on_update=list(si.on_update)
                        )
                out.append(inst)
            blk.instructions = out
    return n_split


def _build_nc(loop_n=None):
    nc = bass.Bass()

    d_qe = nc.declare_dram_parameter("qe", [C, IP], BF16, isOutput=False)
    d_pe = nc.declare_dram_parameter("pe", [C, JQ], BF16, isOutput=False)
    d_ft = nc.declare_dram_parameter("ft", [IP, JQ], BF16, isOutput=False)
    d_qf = nc.declare_dram_parameter("qf", [C, IP], FP16, isOutput=False)
    d_pf = nc.declare_dram_parameter("pf", [C, IP], FP16, isOutput=False)
    d_pmg = nc.declare_dram_parameter("pmg", [C, N], FP16, isOutput=False)
    d_qmg = nc.declare_dram_parameter("qmg", [C, N], FP16, isOutput=False)

    d_rq = nc.declare_dram_parameter("rq", [128, 7 * N], F32, isOutput=True)
    d_w1t = nc.declare_dram_parameter("w1t", [N, IP], F32, isOutput=True)
    d_w2t = nc.declare_dram_parameter("w2t", [N, IP], F32, isOutput=True)

    AOT = mybir.AluOpType

    with tile.TileContext(nc) as tc:
        with (
            tc.tile_pool(name="const", bufs=1) as cp,
            tc.tile_pool(name="pfp", bufs=1) as pfp,
            tc.tile_pool(name="ftp", bufs=2) as ftp,
            tc.tile_pool(name="pr", bufs=3) as prp,
            tc.tile_pool(name="wst", bufs=2) as wstp,
            tc.tile_pool(name="psim", bufs=2, space=bass.MemorySpace.PSUM) as psp,
            tc.tile_pool(name="pw", bufs=1, space=bass.MemorySpace.PSUM) as wpsp,
        ):
            for _rep in range(loop_n or 1):
                # ---- input DMAs: one per tensor (k chunks side by side) ----
                qf_sb = cp.tile([128, KC * IP], FP16, tag="qf")
                nc.sync.dma_start(
                    qf_sb[:].rearrange("p (k x) -> p k x", k=KC),
                    d_qf.rearrange("(k p) x -> p k x", p=128))
                pmg_sb = cp.tile([128, KC * N], FP16, tag="pmg")
                nc.sync.dma_start(
                    pmg_sb[:].rearrange("p (k x) -> p k x", k=KC),
                    d_pmg.rearrange("(k p) x -> p k x", p=128))
                qe_sb = cp.tile([128, KC * IP], BF16, tag="qe")
                nc.sync.dma_start(
                    qe_sb[:].rearrange("p (k x) -> p k x", k=KC),
                    d_qe.rearrange("(k p) x -> p k x", p=128))
                pe_sb = cp.tile([128, KC * JQ], BF16, tag="pe")
                nc.sync.dma_start(
                    pe_sb[:].rearrange("p (k x) -> p k x", k=KC),
                    d_pe.rearrange("(k p) x -> p k x", p=128))

                rq_sb = cp.tile([128, 7 * N], F32, tag="rq")
                w1t_sb = cp.tile([N, IP], F32, tag="w1t")

                # ---- weight_1^T = relu(pmg^T @ qf) + 1e-3 : [N, IP] ----
                for c0 in range(0, IP, 512):
                    w = min(512, IP - c0)
                    ps2 = wpsp.tile([128, 2, 512], F32, tag="wps")
                    ps = ps2[:, 0, :]
                    for k in range(KC):
                        nc.tensor.matmul(
                            ps[:, 0:w],
                            pmg_sb[:, k * N:(k + 1) * N],
                            qf_sb[:, k * IP + c0:k * IP + c0 + w],
                            start=(k == 0),
                            stop=(k == KC - 1),
                        )
                    nc.vector.tensor_scalar(
                        w1t_sb[:, c0:c0 + w], ps[:N, 0:w], 0.001, 0.001,
                        op0=AOT.add, op1=AOT.max,
                    )
                nc.gpsimd.dma_start(d_w1t[:, :], w1t_sb[:])

                # ---- main loop: D[ip, j] = sum_q flow * (dot_raw*rsq*rsp) ----
                row0 = 0
                for mt, rows in enumerate(M_TILES):
                    ft_tile = ftp.tile([128, JQ], BF16, tag="ft")
                    nc.sync.dma_start(
                        ft_tile[0:rows, :], d_ft[row0:row0 + rows, :])
                    for ch0, ng in GROUPS:
                        c0 = ch0 * CHUNK
                        gw = min(ng * CHUNK, JQ - c0)   # 1470 or 392
                        nj = gw // HW
                        ps = psp.tile([128, 3, 512], F32, tag="ps")
                        for gi in range(ng):
                            w = min(CHUNK, JQ - (ch0 + gi) * CHUNK)
                            for k in range(KC):
                                nc.tensor.matmul(
                                    ps[0:rows, gi, 0:w],
                                    qe_sb[:, k * IP + row0:k * IP + row0 + rows],
                                    pe_sb[:, k * JQ + c0 + gi * CHUNK:
                                          k * JQ + c0 + gi * CHUNK + w],
                                    start=(k == 0),
                                    stop=(k == KC - 1),
                                )
                        # prod = psum * flow  (fused, PSUM-direct, one op/group)
                        pr_tile = prp.tile([128, 3 * CHUNK], BF16, tag="pr")
                        if ng > 1:
                            nc.vector.scalar_tensor_tensor(
                                pr_tile[0:rows, 0:gw].rearrange(
                                    "p (g q) -> p g q", q=CHUNK),
                                ps[0:rows, 0:ng, 0:CHUNK],
                                1.0,
                                ft_tile[0:rows, c0:c0 + gw].rearrange(
                                    "p (g q) -> p g q", q=CHUNK),
                                op0=AOT.mult,
                                op1=AOT.mult,
                            )
                        else:
                            nc.vector.scalar_tensor_tensor(
                                pr_tile[0:rows, 0:gw],
                                ps[0:rows, 0, 0:gw],
                                1.0,
                                ft_tile[0:rows, c0:c0 + gw],
                                op0=AOT.mult,
                                op1=AOT.mult,
                            )
                        # sum over q within each j group
                        nc.vector.tensor_reduce(
                            rq_sb[0:rows, mt * N + ch0 * 10:
                                  mt * N + ch0 * 10 + nj],
                            pr_tile[0:rows, 0:gw].rearrange(
                                "p (j q) -> p j q", q=HW),
                            axis=mybir.AxisListType.X,
                            op=AOT.add,
                        )
                    row0 += rows
                nc.gpsimd.dma_start(d_rq[:, :], rq_sb[:])

                # ---- weight_2 (n-sharded): relu(qmg^T @ pf) + 1e-3 : [N, IP] ----
                qmg_sb = cp.tile([128, KC * N], FP16, tag="qmg")
                nc.sync.dma_start(
                    qmg_sb[:].rearrange("p (k x) -> p k x", k=KC),
                    d_qmg.rearrange("(k p) x -> p k x", p=128))
                pf_sb = pfp.tile([128, KC * IP], FP16, tag="pf")
                nc.sync.dma_start(
                    pf_sb[:].rearrange("p (k x) -> p k x", k=KC),
                    d_pf.rearrange("(k p) x -> p k x", p=128))
                w2t_sb = cp.tile([N, IP], F32, tag="w2t")
                for c0 in range(0, IP, 512):
                    w = min(512, IP - c0)
                    ps2 = wpsp.tile([128, 2, 512], F32, tag="wps")
                    ps = ps2[:, 1, :]
                    for k in range(KC):
                        nc.tensor.matmul(
                            ps[:, 0:w],
                            qmg_sb[:, k * N:(k + 1) * N],
                            pf_sb[:, k * IP + c0:k * IP + c0 + w],
                            start=(k == 0),
                            stop=(k == KC - 1),
                        )
                    nc.vector.tensor_scalar(
                        w2t_sb[:, c0:c0 + w], ps[:N, 0:w], 0.001, 0.001,
                        op0=AOT.add, op1=AOT.max,
                    )
                nc.gpsimd.dma_start(d_w2t[:, :], w2t_sb[:])

    _split_waits(nc)
    return nc


def _get_nc(loop_n=None):
    key = ("nc", loop_n)
    if key not in _BUILT:
        _BUILT[key] = _build_nc(loop_n)
    return _BUILT[key]


def kernel(query, proto, flow):
    global LAST_RESULT
    query = np.asarray(query, dtype=np.float32)
    proto = np.asarray(proto, dtype=np.float32)
    flow = np.asarray(flow, dtype=np.float32)

    qr = query.reshape(M, C, HW)
    pr = proto.reshape(N, C, HW)

    # channel-centering stats (match reference fp32 math)
    qm = qr.mean(axis=1)                      # [M, HW]
    pm = pr.mean(axis=1)                      # [N, HW]
    qc = qr - qm[:, None, :]
    pc = pr - pm[:, None, :]
    nq = np.sqrt((qc * qc).sum(axis=1))       # [M, HW]
    npn = np.sqrt((pc * pc).sum(axis=1))      # [N, HW]
    rsq = 1.0 / np.maximum(nq, COS_EPS)
    rsp = 1.0 / np.maximum(npn, COS_EPS)

    Bp = pr.mean(axis=2)                      # [N, C]  GAP(proto)
    Ap = qr.mean(axis=2)                      # [M, C]  GAP(query)

    # packed contraction operands: psum = qe^T @ pe = dot_raw*rsq*rsp
    qe = np.ascontiguousarray(
        (qr * rsq[:, None, :]).transpose(1, 0, 2).reshape(C, M * HW)
        .astype(NP_BF16))
    pe = np.ascontiguousarray(
        (pr * rsp[:, None, :]).transpose(1, 0, 2).reshape(C, JQ)
        .astype(NP_BF16))

    # F3[i,j] = sum_pq flow * (1 + C*qm*pm*rsq*rsp)
    a_w = C * qm * rsq                        # [M, HW]
    b_w = pm * rsp                            # [N, HW]
    F1 = flow.sum(axis=(2, 3), dtype=np.float64)                     # [M, N]
    Fc = np.einsum('ijpq,ip,jq->ij', flow, a_w, b_w, optimize=True)  # [M, N]
    F3 = F1 + Fc

    qf_full = qr.transpose(1, 0, 2).reshape(C, M * HW).astype(NP_FP16)
    pf_full = pr.transpose(1, 0, 2).reshape(C, JQ).astype(NP_FP16)
    pmg = np.ascontiguousarray(Bp.T.astype(NP_FP16))
    qmg = np.ascontiguousarray(Ap.T.astype(NP_FP16))     # [C, M] full

    in_maps = []
    for cc in range(NCORES):
        cols = slice(cc * IP, (cc + 1) * IP)
        ft_cc = (
            flow[cc * MS:(cc + 1) * MS]
            .transpose(0, 2, 1, 3)
            .astype(NP_BF16)
            .reshape(IP, JQ)
        )
        in_maps.append({
            "qe": np.ascontiguousarray(qe[:, cols]),
            "pe": pe,
            "ft": ft_cc,
            "qf": np.ascontiguousarray(qf_full[:, cols]),
            "pf": np.ascontiguousarray(pf_full[:, cols]),
            "pmg": pmg,
            "qmg": qmg,
        })

    nc = _get_nc()
    res = run_bass_kernel_spmd(nc, in_maps, list(range(NCORES)))
    LAST_RESULT = res

    logits = np.empty((M, N), np.float32)
    w1 = np.empty((M, N, HW), np.float32)
    w2 = np.empty((N, M, HW), np.float32)
    for cc in range(NCORES):
        out = res.results[cc]
        rq = out["rq"]
        big = np.zeros((IP, N), np.float64)
        r0 = 0
        for mt, rows in enumerate(M_TILES):
            big[r0:r0 + rows] = rq[:rows, mt * N:(mt + 1) * N]
            r0 += rows
        D = big.reshape(MS, HW, N).sum(axis=1)          # [MS, N]
        logits[cc * MS:(cc + 1) * MS] = (
            (F3[cc * MS:(cc + 1) * MS] - D) * (TEMP / HW)
        ).astype(np.float32)
        w1[cc * MS:(cc + 1) * MS] = (
            out["w1t"].reshape(N, MS, HW).transpose(1, 0, 2)
        )
        w2[cc * MS:(cc + 1) * MS, :, :] = (
            out["w2t"].reshape(N, MS, HW).transpose(1, 0, 2)
        )
    return logits, w1, w2


# revision 24
# speedup vs baseline: 40.1016x; 1.6660x over previous
"""DeepEMD kernel for 8x Trainium2 NeuronCores (Bass/Tile).

Computes (logits, weight_1, weight_2) of the DeepEMD head:
  weight_1 = relu(einsum('mchw,nc->mnhw', query, GAP(proto))) + 0.001
  weight_2 = relu(einsum('nchw,mc->nmhw', proto, GAP(query))) + 0.001
  sim      = cosine sim over channels of centered features  [M,N,HW,HW]
  logits   = ((1-sim)*flow).sum((-1,-2)) * (T/HW)

Sharding: M (query rows) split across 8 cores; proto replicated.

Device strategy (per core):
  Host packs qe = (q^T*rsq) [640,784] bf16 and pe = (p^T*rsp) [640,6272]
  bf16 so psum[ip,jq] = qe^T@pe = dot_raw*rsq*rsp.  With the identity
     sum_pq flow*(1-sim) = F3 - sum_pq flow*dot_raw*rsq*rsp,
     F3[i,j] = sum_pq flow*(1 + C*qm*pm*rsq*rsp)   (host, tiny),
  the device only needs the raw 640-deep contraction, one fused
  DVE multiply-by-flow straight out of PSUM (3 banks per op), and a
  grouped q-reduce; p-sums + F3 finish on host.
  weight_1/weight_2 are small fp16 matmuls with fused max(x+1e-3, 1e-3).

  Empirical notes for this environment (measured via in-NEFF repeat
  loops): PE matmuls run at warm spec (~N/2.4GHz), DMA ~280 GB/s/core,
  but every DVE/ACT instruction and its semaphore waits carry multi-us
  overheads -> the design minimizes instruction count (batched 3-bank
  PSUM consumers, one DMA per input tensor).
"""

import os
import sys

if "/opt/trn_rl_repo" not in sys.path:
    sys.path.insert(0, "/opt/trn_rl_repo")

# NTFF tracing is unavailable in this container (antenv.axon_hooks missing);
# make sure run_bass_kernel_spmd never takes the trace path even if the
# caller's environment sets BASS_TRACE.
os.environ["BASS_NEVER_TRACE"] = "1"

import ml_dtypes
import numpy as np

import concourse.bass as bass
import concourse.tile as tile
from concourse import mybir
from concourse.bass_utils import run_bass_kernel_spmd

# Problem shape (hardcoded per contest rules)
M = 128          # queries
N = 128          # protos
C = 640          # channels
HW = 49          # spatial
NCORES = 8
MS = M // NCORES          # 16 queries per core
IP = MS * HW              # 784 query rows (i,p) per core
JQ = N * HW               # 6272 proto cols (j,q)
KC = C // 128             # 5 contraction chunks of 128
TEMP = 12.5
COS_EPS = 1e-8

CHUNK = 490               # 10 j's per psum bank (q-group aligned)
# chunk groups: each group is one [128, g, 512] psum tile (g banks), with
# one fused DVE multiply + one grouped reduce per group
GROUPS = [(0, 3), (3, 3), (6, 3), (9, 3), (12, 1)]   # (chunk0, nchunks)
M_TILES = [128] * 6 + [IP - 6 * 128]                 # 784 = 6*128 + 16

BF16 = mybir.dt.bfloat16
FP8 = mybir.dt.float8e4
FP16 = mybir.dt.float16
F32 = mybir.dt.float32

NP_BF16 = ml_dtypes.bfloat16
NP_FP16 = np.float16
NP_FP8 = ml_dtypes.float8_e4m3

_BUILT = {}
LAST_RESULT = None        # test harness reads exec_time_ns from here

# walrus embeds sem-waits directly in the ISA structs, which have very few
# wait slots. Tile can attach more than that. Spill the excess onto
# standalone InstEventSemaphore waits inserted just before the offending
# instruction on the same engine stream.
_WAIT_LIMIT = 1


def _split_waits(nc):
    n_split = 0
    for fn in nc.m.functions:
        for blk in fn.blocks:
            out = []
            for inst in blk.instructions:
                si = inst.sync_info
                tn = type(inst).__name__
                if si is not None and tn != "InstEventSemaphore":
                    waits = list(si.on_wait)
                    if len(waits) > _WAIT_LIMIT:
                        spill, keep = waits[:-_WAIT_LIMIT], waits[-_WAIT_LIMIT:]
                        for w in spill:
                            n_split += 1
                            out.append(mybir.InstEventSemaphore(
                                name=f"{inst.name}-wsp{n_split}",
                                engine=inst.engine,
                                sync_info=mybir.SyncInfo(on_wait=[w], on_update=[]),
                            ))
                        inst.sync_info = mybir.SyncInfo(
                            on_wait=keep, on_update=list(si.on_update)
                        )
                out.append(inst)
            blk.instructions = out
    return n_split


def _build_nc(loop_n=None, red_mt=True):
    nc = bass.Bass()

    d_qe = nc.declare_dram_parameter("qe", [C, IP], BF16, isOutput=False)
    d_pe = nc.declare_dram_parameter("pe", [C, JQ], BF16, isOutput=False)
    d_ft = nc.declare_dram_parameter("ft", [IP, JQ], BF16, isOutput=False)
    d_qf = nc.declare_dram_parameter("qf", [C, IP], FP16, isOutput=False)
    d_pf = nc.declare_dram_parameter("pf", [C, IP], FP16, isOutput=False)
    d_pmg = nc.declare_dram_parameter("pmg", [C, N], FP16, isOutput=False)
    d_qmg = nc.declare_dram_parameter("qmg", [C, N], FP16, isOutput=False)

    d_rq = nc.declare_dram_parameter("rq", [128, 7 * N], F32, isOutput=True)
    d_w1t = nc.declare_dram_parameter("w1t", [N, IP], F32, isOutput=True)
    d_w2t = nc.declare_dram_parameter("w2t", [N, IP], F32, isOutput=True)

    AOT = mybir.AluOpType

    with tile.TileContext(nc) as tc:
        with (
            tc.tile_pool(name="const", bufs=1) as cp,
            tc.tile_pool(name="pfp", bufs=1) as pfp,
            tc.tile_pool(name="ftp", bufs=2) as ftp,
            tc.tile_pool(name="pr", bufs=3) as prp,
            tc.tile_pool(name="wst", bufs=2) as wstp,
            tc.tile_pool(name="psim", bufs=2, space=bass.MemorySpace.PSUM) as psp,
            tc.tile_pool(name="pw", bufs=1, space=bass.MemorySpace.PSUM) as wpsp,
        ):
            for _rep in range(loop_n or 1):
                # ---- input DMAs: one per tensor (k chunks side by side) ----
                qf_sb = cp.tile([128, KC * IP], FP16, tag="qf")
                nc.sync.dma_start(
                    qf_sb[:].rearrange("p (k x) -> p k x", k=KC),
                    d_qf.rearrange("(k p) x -> p k x", p=128))
                pmg_sb = cp.tile([128, KC * N], FP16, tag="pmg")
                nc.sync.dma_start(
                    pmg_sb[:].rearrange("p (k x) -> p k x", k=KC),
                    d_pmg.rearrange("(k p) x -> p k x", p=128))
                qe_sb = cp.tile([128, KC * IP], BF16, tag="qe")
                nc.sync.dma_start(
                    qe_sb[:].rearrange("p (k x) -> p k x", k=KC),
                    d_qe.rearrange("(k p) x -> p k x", p=128))
                PE_A = 6 * CHUNK                     # 2940 (chunks 0-5)
                PE_B = JQ - PE_A                     # 3332 (chunks 6-12)
                pe_a = cp.tile([128, KC * PE_A], BF16, tag="pea")
                nc.sync.dma_start(
                    pe_a[:].rearrange("p (k x) -> p k x", k=KC),
                    d_pe[:, 0:PE_A].rearrange("(k p) x -> p k x", p=128))
                pe_b = cp.tile([128, KC * PE_B], BF16, tag="peb")
                nc.sync.dma_start(
                    pe_b[:].rearrange("p (k x) -> p k x", k=KC),
                    d_pe[:, PE_A:JQ].rearrange("(k p) x -> p k x", p=128))

                rq_sb = cp.tile([128, 7 * N], F32, tag="rq")
                w1t_sb = cp.tile([N, IP], F32, tag="w1t")

                # ---- weight_1^T = relu(pmg^T @ qf) + 1e-3 : [N, IP] ----
                for c0 in range(0, IP, 512):
                    w = min(512, IP - c0)
                    ps2 = wpsp.tile([128, 2, 512], F32, tag="wps")
                    ps = ps2[:, 0, :]
                    for k in range(KC):
                        nc.tensor.matmul(
                            ps[:, 0:w],
                            pmg_sb[:, k * N:(k + 1) * N],
                            qf_sb[:, k * IP + c0:k * IP + c0 + w],
                            start=(k == 0),
                            stop=(k == KC - 1),
                        )
                    nc.vector.tensor_scalar(
                        w1t_sb[:, c0:c0 + w], ps[:N, 0:w], 0.001, 0.001,
                        op0=AOT.add, op1=AOT.max,
                    )
                nc.gpsimd.dma_start(d_w1t[:, :], w1t_sb[:])

                # ---- main loop: D[ip, j] = sum_q flow * (dot_raw*rsq*rsp) ----
                row0 = 0
                for mt, rows in enumerate(M_TILES):
                    ft_tile = ftp.tile([128, JQ], BF16, tag="ft")
                    nc.sync.dma_start(
                        ft_tile[0:rows, :], d_ft[row0:row0 + rows, :])
                    for ch0, ng in GROUPS:
                        c0 = ch0 * CHUNK
                        gw = min(ng * CHUNK, JQ - c0)   # 1470 or 392
                        nj = gw // HW
                        ps = psp.tile([128, 3, 512], F32, tag="ps")
                        for gi in range(ng):
                            w = min(CHUNK, JQ - (ch0 + gi) * CHUNK)
                            ch = ch0 + gi
                            if ch < 6:
                                src_t, W_h, lo = pe_a, PE_A, ch * CHUNK
                            else:
                                src_t, W_h, lo = pe_b, PE_B, ch * CHUNK - PE_A
                            for k in range(KC):
                                nc.tensor.matmul(
                                    ps[0:rows, gi, 0:w],
                                    qe_sb[:, k * IP + row0:k * IP + row0 + rows],
                                    src_t[:, k * W_h + lo:k * W_h + lo + w],
                                    start=(k == 0),
                                    stop=(k == KC - 1),
                                )
                        # prod = psum * flow  (fused, PSUM-direct, one op/group)
                        if ch0 == 0:
                            pr_mt = prp.tile([128, JQ], BF16, tag="pr")
                        pr_tile = pr_mt[:, c0:c0 + 3 * CHUNK] if ch0 < 12 \
                            else pr_mt[:, c0:JQ]
                        if ng > 1:
                            nc.vector.scalar_tensor_tensor(
                                pr_tile[0:rows, 0:gw].rearrange(
                                    "p (g q) -> p g q", q=CHUNK),
                                ps[0:rows, 0:ng, 0:CHUNK],
                                1.0,
                                ft_tile[0:rows, c0:c0 + gw].rearrange(
                                    "p (g q) -> p g q", q=CHUNK),
                                op0=AOT.mult,
                                op1=AOT.mult,
                            )
                        else:
                            nc.vector.scalar_tensor_tensor(
                                pr_tile[0:rows, 0:gw],
                                ps[0:rows, 0, 0:gw],
                                1.0,
                                ft_tile[0:rows, c0:c0 + gw],
                                op0=AOT.mult,
                                op1=AOT.mult,
                            )
                        # sum over q within each j group
                        if not red_mt:
                            nc.vector.tensor_reduce(
                                rq_sb[0:rows, mt * N + ch0 * 10:
                                      mt * N + ch0 * 10 + nj],
                                pr_tile[0:rows, 0:gw].rearrange(
                                    "p (j q) -> p j q", q=HW),
                                axis=mybir.AxisListType.X,
                                op=AOT.add,
                            )
                    if red_mt:
                        nc.vector.tensor_reduce(
                            rq_sb[0:rows, mt * N:mt * N + N],
                            pr_mt[0:rows, :].rearrange("p (j q) -> p j q", q=HW),
                            axis=mybir.AxisListType.X,
                            op=AOT.add,
                        )
                    row0 += rows
                nc.gpsimd.dma_start(d_rq[:, :], rq_sb[:])

                # ---- weight_2 (n-sharded): relu(qmg^T @ pf) + 1e-3 : [N, IP] ----
                qmg_sb = cp.tile([128, KC * N], FP16, tag="qmg")
                nc.sync.dma_start(
                    qmg_sb[:].rearrange("p (k x) -> p k x", k=KC),
                    d_qmg.rearrange("(k p) x -> p k x", p=128))
                pf_sb = pfp.tile([128, KC * IP], FP16, tag="pf")
                nc.sync.dma_start(
                    pf_sb[:].rearrange("p (k x) -> p k x", k=KC),
                    d_pf.rearrange("(k p) x -> p k x", p=128))
                w2t_sb = cp.tile([N, IP], F32, tag="w2t")
                for c0 in range(0, IP, 512):
                    w = min(512, IP - c0)
                    ps2 = wpsp.tile([128, 2, 512], F32, tag="wps")
                    ps = ps2[:, 1, :]
                    for k in range(KC):
                        nc.tensor.matmul(
                            ps[:, 0:w],
                            qmg_sb[:, k * N:(k + 1) * N],
                            pf_sb[:, k * IP + c0:k * IP + c0 + w],
                            start=(k == 0),
                            stop=(k == KC - 1),
                        )
                    nc.vector.tensor_scalar(
                        w2t_sb[:, c0:c0 + w], ps[:N, 0:w], 0.001, 0.001,
                        op0=AOT.add, op1=AOT.max,
                    )
                nc.gpsimd.dma_start(d_w2t[:, :], w2t_sb[:])

    _split_waits(nc)
    return nc


def _get_nc(loop_n=None, red_mt=True):
    key = ("nc", loop_n, red_mt)
    if key not in _BUILT:
        _BUILT[key] = _build_nc(loop_n, red_mt)
    return _BUILT[key]


def kernel(query, proto, flow):
    global LAST_RESULT
    query = np.asarray(query, dtype=np.float32)
    proto = np.asarray(proto, dtype=np.float32)
    flow = np.asarray(flow, dtype=np.float32)

    qr = query.reshape(M, C, HW)
    pr = proto.reshape(N, C, HW)

    # channel-centering stats (match reference fp32 math)
    qm = qr.mean(axis=1)                      # [M, HW]
    pm = pr.mean(axis=1)                      # [N, HW]
    qc = qr - qm[:, None, :]
    pc = pr - pm[:, None, :]
    nq = np.sqrt((qc * qc).sum(axis=1))       # [M, HW]
    npn = np.sqrt((pc * pc).sum(axis=1))      # [N, HW]
    rsq = 1.0 / np.maximum(nq, COS_EPS)
    rsp = 1.0 / np.maximum(npn, COS_EPS)

    Bp = pr.mean(axis=2)                      # [N, C]  GAP(proto)
    Ap = qr.mean(axis=2)                      # [M, C]  GAP(query)

    # packed contraction operands: psum = qe^T @ pe = dot_raw*rsq*rsp
    qe = np.ascontiguousarray(
        (qr * rsq[:, None, :]).transpose(1, 0, 2).reshape(C, M * HW)
        .astype(NP_BF16))
    pe = np.ascontiguousarray(
        (pr * rsp[:, None, :]).transpose(1, 0, 2).reshape(C, JQ)
        .astype(NP_BF16))

    # F3[i,j] = sum_pq flow * (1 + C*qm*pm*rsq*rsp)
    a_w = C * qm * rsq                        # [M, HW]
    b_w = pm * rsp                            # [N, HW]
    F1 = flow.sum(axis=(2, 3), dtype=np.float64)                     # [M, N]
    Fc = np.einsum('ijpq,ip,jq->ij', flow, a_w, b_w, optimize=True)  # [M, N]
    F3 = F1 + Fc

    qf_full = qr.transpose(1, 0, 2).reshape(C, M * HW).astype(NP_FP16)
    pf_full = pr.transpose(1, 0, 2).reshape(C, JQ).astype(NP_FP16)
    pmg = np.ascontiguousarray(Bp.T.astype(NP_FP16))
    qmg = np.ascontiguousarray(Ap.T.astype(NP_FP16))     # [C, M] full

    in_maps = []
    for cc in range(NCORES):
        cols = slice(cc * IP, (cc + 1) * IP)
        ft_cc = (
            flow[cc * MS:(cc + 1) * MS]
            .transpose(0, 2, 1, 3)
            .astype(NP_BF16)
            .reshape(IP, JQ)
        )
        in_maps.append({
            "qe": np.ascontiguousarray(qe[:, cols]),
            "pe": pe,
            "ft": ft_cc,
            "qf": np.ascontiguousarray(qf_full[:, cols]),
            "pf": np.ascontiguousarray(pf_full[:, cols]),
            "pmg": pmg,
            "qmg": qmg,
        })

    nc = _get_nc()
    res = run_bass_kernel_spmd(nc, in_maps, list(range(NCORES)))
    LAST_RESULT = res

    logits = np.empty((M, N), np.float32)
    w1 = np.empty((M, N, HW), np.float32)
    w2 = np.empty((N, M, HW), np.float32)
    for cc in range(NCORES):
        out = res.results[cc]
        rq = out["rq"]
        big = np.zeros((IP, N), np.float64)
        r0 = 0
        for mt, rows in enumerate(M_TILES):
            big[r0:r0 + rows] = rq[:rows, mt * N:(mt + 1) * N]
            r0 += rows
        D = big.reshape(MS, HW, N).sum(axis=1)          # [MS, N]
        logits[cc * MS:(cc + 1) * MS] = (
            (F3[cc * MS:(cc + 1) * MS] - D) * (TEMP / HW)
        ).astype(np.float32)
        w1[cc * MS:(cc + 1) * MS] = (
            out["w1t"].reshape(N, MS, HW).transpose(1, 0, 2)
        )
        w2[cc * MS:(cc + 1) * MS, :, :] = (
            out["w2t"].reshape(N, MS, HW).transpose(1, 0, 2)
        )
    return logits, w1, w2
